# revision 1
# baseline (speedup 1.0000x reference)
"""Conformer encoder layer on 8 Trainium2 NeuronCores (Bass/Tile).

Sharding: data-parallel over batch N=16 -> 2 batch elements per core, no
collectives. Per-core activations live in channel-major layout [D, T] with
T = n_local*512 + l (each batch's sequence contiguous), which keeps every
matmul in lhsT.T @ rhs form without activation transposes.

Key techniques:
  - LayerNorm gammas and module biases folded into weights host-side;
    mean/E[x^2] computed via (1/D)-matmuls on the PE whose output is
    pre-broadcast to all 128 partitions, applied with DVE tensor ops.
  - rel_shift as one diagonal-AP SBUF->SBUF DMA per score tile:
    out[l', j] = band[l', 127 - l' + j] is affine in flat SBUF space.
  - softmax normalization fused into the score-transpose matmul: the
    transpose of exp-scores is a matmul with diag(1/Z) as moving operand.
  - depthwise conv as 31 shifted multiply-accumulate ops split across the
    vector and gpsimd engines; BatchNorm folded into the conv weights.
"""
from contextlib import ExitStack

import numpy as np

import concourse.bass as bass
import concourse.bacc as bacc
import concourse.tile as tile
import concourse.mybir as mybir
from concourse.bass_utils import run_bass_kernel_spmd

dt = mybir.dt
AF = mybir.ActivationFunctionType
OP = mybir.AluOpType
ts = bass.ts
f32 = dt.float32
F32R = True               # use fp32r (bf16-pair) matmuls: ~4x PE throughput
REPEAT = 1                # duplicate whole program (timing experiments)
STAGES = 99               # truncate program after N stages (timing experiments)
mdt = dt.float32r if F32R else dt.float32

D, H, DH, F, KW, L, N = 512, 8, 64, 2048, 31, 512, 16
EPS = 1e-5
NCORES = 8
NB = N // NCORES          # batches per core (2)
T = NB * L                # tokens per core (1024)
DT = D // 128             # channel tiles (4)
FT = F // 128             # FFN hidden tiles (16)
TC = T // 512             # 512-token chunks (2)
PAD = KW // 2             # conv padding (15)
SCALE = DH ** -0.5        # 0.125


def _build():
    nc = bacc.Bacc("TRN2", target_bir_lowering=False, debug=False,
                   num_devices=NCORES)

    def I(name, shape, d=f32):
        return nc.dram_tensor(name, list(shape), d, kind="ExternalInput").ap()

    dd = {
        "x": I("x", (D, T), mdt),
        "posT": I("posT", (D, 1024), mdt),
        "winT": I("winT", (D, 3 * D), mdt),
        "bqu": I("bqu", (128, DT)), "bqv": I("bqv", (128, DT)),
        "bva": I("bva", (128, DT)),
        "woT": I("woT", (D, D), mdt), "bo": I("bo", (128, DT)),
        "pwT": I("pwT", (D, D), mdt),
        "wc1T": I("wc1T", (D, 2 * D), mdt), "bc1": I("bc1", (128, 2 * DT)),
        "wdw": I("wdw", (128, DT * KW)), "bdw": I("bdw", (128, DT)),
        "wc2T": I("wc2T", (D, D), mdt), "bc2": I("bc2", (128, DT)),
        "gf": I("gf", (128, DT)), "bf": I("bf", (128, DT)),
    }
    for m in ("ffm", "ff"):
        dd[f"w1T_{m}"] = I(f"w1T_{m}", (D, F), mdt)
        dd[f"b1_{m}"] = I(f"b1_{m}", (128, FT))
        dd[f"w2T_{m}"] = I(f"w2T_{m}", (F, D), mdt)
        dd[f"b2_{m}"] = I(f"b2_{m}", (1, D), mdt)
    y_d = nc.dram_tensor("y", [D, T], f32, kind="ExternalOutput").ap()

    with tile.TileContext(nc) as tc:
        for _rep in range(REPEAT):
            _emit(nc, tc, dd, y_d)
    nc.compile()
    return nc


def _emit(nc, tc, dd, y_d):
    ctx = ExitStack()
    with ctx:
        root = ctx.enter_context(tc.tile_pool(name="root", bufs=1))

        x = [root.tile([128, T], mdt, tag=f"x{j}", name=f"x{j}", bufs=1) for j in range(DT)]
        for j in range(DT):
            nc.sync.dma_start(x[j][:], dd["x"][ts(j, 128), :])
        ones = root.tile([128, 128], mdt, tag="ones", name="ones", bufs=1)
        onesrow = root.tile([1, 512], mdt, tag="onesrow", name="onesrow", bufs=1)
        cst = root.tile([128, 512], f32, tag="cst", name="cst", bufs=1)
        nc.vector.memset(cst[:], 1.0 / D)
        nc.scalar.copy(ones[:], cst[:, :128])
        nc.vector.memset(cst[:1, :], 1.0)
        nc.scalar.copy(onesrow[:], cst[:1, :])
        idb = root.tile([128, 128], dt.bfloat16, tag="idb", name="idb", bufs=1)
        nc.vector.memset(cst[:, :1], 1.0)
        nc.gpsimd.affine_select(idb[:], cst[:, 0:1].broadcast_to([128, 128]),
                                pattern=[[1, 128]], compare_op=OP.is_equal,
                                fill=0.0, base=0, channel_multiplier=-1)
        xln = [root.tile([128, T], mdt, tag=f"xln{j}", name=f"xln{j}", bufs=1) for j in range(DT)]
        s_b = root.tile([128, T], mdt, tag="s_b", name="s_b", bufs=1)
        ms_b = root.tile([128, T], mdt, tag="ms_b", name="ms_b", bufs=1)

        # ---------------- layernorm: stats + apply -> xln ------------------
        def layer_norm():
            with tc.tile_pool(name="lnp", bufs=1, space="PSUM") as lnps, \
                 tc.tile_pool(name="lns", bufs=2) as lns:
                mp = lnps.tile([128, T], f32, tag="m", name="m", bufs=1)
                qp = lnps.tile([128, T], f32, tag="q", name="q", bufs=1)
                for kc in range(DT):
                    x2 = lns.tile([128, T], mdt, tag="x2", name="x2", bufs=2)
                    nc.scalar.square(x2[:], x[kc][:])
                    for t in range(TC):
                        nc.tensor.matmul(mp[:, ts(t, 512)], ones[:],
                                         x[kc][:, ts(t, 512)],
                                         start=kc == 0, stop=kc == DT - 1,
                                         skip_group_check=True)
                        nc.tensor.matmul(qp[:, ts(t, 512)], ones[:],
                                         x2[:, ts(t, 512)],
                                         start=kc == 0, stop=kc == DT - 1,
                                         skip_group_check=True)
                msq = lns.tile([128, T], f32, tag="tmp", name="tmp", bufs=3)
                nc.scalar.square(msq[:], mp[:])
                veps = lns.tile([128, T], f32, tag="tmp", name="tmp", bufs=3)
                nc.vector.scalar_tensor_tensor(veps[:], qp[:], EPS, msq[:],
                                               op0=OP.add, op1=OP.subtract)
                rec = lns.tile([128, T], f32, tag="tmp", name="tmp", bufs=3)
                nc.vector.reciprocal(rec[:], veps[:])
                nc.scalar.sqrt(s_b[:], rec[:])
                nc.vector.tensor_mul(ms_b[:], mp[:], s_b[:])
                for kc in range(DT):
                    u = lns.tile([128, T], f32, tag="tmp", name="tmp", bufs=3)
                    eng = nc.vector if kc < 2 else nc.gpsimd
                    eng.tensor_mul(u[:], x[kc][:], s_b[:])
                    eng.tensor_sub(xln[kc][:], u[:], ms_b[:])

        def load_wtiles(pool, w_d, ncols, tag):
            w = [pool.tile([128, ncols], mdt, tag=f"{tag}{j}", name=f"{tag}{j}", bufs=1)
                 for j in range(DT)]
            for j in range(DT):
                nc.sync.dma_start(w[j][:], w_d[ts(j, 128), :])
            return w

        # ---------------- FFN ---------------------------------------------
        def ffn(mod):
            layer_norm()
            with tc.tile_pool(name=f"h_{mod}", bufs=1) as hpool, \
                 tc.tile_pool(name=f"ps_{mod}", bufs=1, space="PSUM") as ps:
                h = [hpool.tile([128, T], mdt, tag=f"h{f}", name=f"h{f}", bufs=1)
                     for f in range(FT)]
                with tc.tile_pool(name=f"w1_{mod}", bufs=1) as w1p:
                    w1 = load_wtiles(w1p, dd[f"w1T_{mod}"], F, "w1")
                    b1 = w1p.tile([128, FT], f32, tag="b1", name="b1", bufs=1)
                    nc.sync.dma_start(b1[:], dd[f"b1_{mod}"])
                    for mf in range(FT):
                        for t in range(TC):
                            hp = ps.tile([128, 512], f32, tag="hp", name="hp", bufs=2)
                            for kc in range(DT):
                                nc.tensor.matmul(hp[:], w1[kc][:, ts(mf, 128)],
                                                 xln[kc][:, ts(t, 512)],
                                                 start=kc == 0, stop=kc == DT - 1)
                            nc.scalar.activation(h[mf][:, ts(t, 512)], hp[:],
                                                 AF.Silu, bias=b1[:, mf:mf + 1])
                with tc.tile_pool(name=f"w2_{mod}", bufs=1) as w2p:
                    w2 = [w2p.tile([128, D], mdt, tag=f"w2{f}", name=f"w2{f}", bufs=1)
                          for f in range(FT)]
                    for f in range(FT):
                        nc.sync.dma_start(w2[f][:], dd[f"w2T_{mod}"][ts(f, 128), :])
                    b2 = w2p.tile([1, D], mdt, tag="b2", name="b2", bufs=1)
                    nc.sync.dma_start(b2[:], dd[f"b2_{mod}"])
                    for md in range(DT):
                        yp = [ps.tile([128, 512], f32, tag="yp", name="yp", bufs=2)
                              for _ in range(TC)]
                        for kf in range(FT):
                            for t in range(TC):
                                nc.tensor.matmul(yp[t][:], w2[kf][:, ts(md, 128)],
                                                 h[kf][:, ts(t, 512)],
                                                 start=kf == 0, stop=False,
                                                 skip_group_check=True)
                        for t in range(TC):
                            nc.tensor.matmul(yp[t][:], b2[:, ts(md, 128)],
                                             onesrow[:], start=False, stop=True,
                                             skip_group_check=True)
                            nc.vector.scalar_tensor_tensor(
                                x[md][:, ts(t, 512)], yp[t][:], 0.5,
                                x[md][:, ts(t, 512)], op0=OP.mult, op1=OP.add)

        # =========================== program ===============================
        stage = [0]
        def go():
            stage[0] += 1
            return STAGES >= stage[0]

        p = [root.tile([128, 1024], mdt, tag=f"p{j}", name=f"p{j}", bufs=1) for j in range(DT)]
        if go():  # S1 pos
            with tc.tile_pool(name="pos", bufs=1) as pospool, \
                 tc.tile_pool(name="posps", bufs=2, space="PSUM") as posps:
                posT = load_wtiles(pospool, dd["posT"], 1024, "posT")
                pwT = load_wtiles(pospool, dd["pwT"], D, "pwT")
                for mf in range(DT):
                    for t in range(2):
                        pp = posps.tile([128, 512], f32, tag="pp", name="pp", bufs=2)
                        for kc in range(DT):
                            nc.tensor.matmul(pp[:], pwT[kc][:, ts(mf, 128)],
                                             posT[kc][:, ts(t, 512)],
                                             start=kc == 0, stop=kc == DT - 1)
                        nc.scalar.copy(p[mf][:, ts(t, 512)], pp[:])

        if go():  # S2 ffn1
            ffn("ffm")

        # ---------------- attention ---------------------------------------
        layer_norm()
        with tc.tile_pool(name="attn", bufs=1) as ap:
            q_u = [ap.tile([128, T], mdt, tag=f"qu{j}", name=f"qu{j}", bufs=1) for j in range(DT)]
            q_v = [ap.tile([128, T], mdt, tag=f"qv{j}", name=f"qv{j}", bufs=1) for j in range(DT)]
            ksb = [ap.tile([128, T], mdt, tag=f"k{j}", name=f"k{j}", bufs=1) for j in range(DT)]
            vT = [ap.tile([128, D], mdt, tag=f"vT{t}", name=f"vT{t}", bufs=1) for t in range(8)]
            att = [ap.tile([128, T], mdt, tag=f"att{j}", name=f"att{j}", bufs=1) for j in range(DT)]
            bqu = ap.tile([128, DT], f32, tag="bqu", name="bqu", bufs=1)
            bqv = ap.tile([128, DT], f32, tag="bqv", name="bqv", bufs=1)
            bva = ap.tile([128, DT], f32, tag="bva", name="bva", bufs=1)
            for t_, nm in ((bqu, "bqu"), (bqv, "bqv"), (bva, "bva")):
                nc.sync.dma_start(t_[:], dd[nm])

            if go():  # S4 qkv
                with tc.tile_pool(name="win", bufs=1) as wp, \
                     tc.tile_pool(name="qkvps", bufs=2, space="PSUM") as qps:
                    win = load_wtiles(wp, dd["winT"], 3 * D, "win")
                    for mf in range(2 * DT):          # q then k channel tiles
                        for t in range(TC):
                            qp = qps.tile([128, 512], f32, tag="qp", name="qp", bufs=2)
                            for kc in range(DT):
                                nc.tensor.matmul(qp[:], win[kc][:, ts(mf, 128)],
                                                 xln[kc][:, ts(t, 512)],
                                                 start=kc == 0, stop=kc == DT - 1)
                            if mf < DT:
                                nc.scalar.activation(q_u[mf][:, ts(t, 512)], qp[:],
                                                     AF.Identity, bias=bqu[:, mf:mf + 1])
                                nc.scalar.activation(q_v[mf][:, ts(t, 512)], qp[:],
                                                     AF.Identity, bias=bqv[:, mf:mf + 1])
                            else:
                                nc.scalar.copy(ksb[mf - DT][:, ts(t, 512)], qp[:])
                    for tt in range(8):               # v, transposed layout
                        vp = qps.tile([128, 512], f32, tag="qp", name="qp", bufs=2)
                        for kc in range(DT):
                            nc.tensor.matmul(vp[:], xln[kc][:, ts(tt, 128)],
                                             win[kc][:, 2 * D:3 * D],
                                             start=kc == 0, stop=kc == DT - 1)
                        nc.scalar.copy(vT[tt][:], vp[:])

            if go():  # S5 scores
                with tc.tile_pool(name="sc", bufs=1) as sc, \
                     tc.tile_pool(name="scps", bufs=1, space="PSUM") as scps:
                    for n in range(NB):
                        for hpair in range(H // 2):
                            aos = [scps.tile([128, 512], f32, tag="ao", name="ao",
                                             bufs=2) for _ in range(2)]
                            for h2 in range(2):
                                ao = aos[h2]
                                h_ = 2 * hpair + h2
                                mt, base = h_ // 2, (h_ % 2) * 64
                                qu_h = q_u[mt][base:base + 64, :]
                                qv_h = q_v[mt][base:base + 64, :]
                                k_h = ksb[mt][base:base + 64, :]
                                p_h = p[mt][base:base + 64, :]
                                probs = []
                                for bl in range(4):
                                    lsl = ts(n * 4 + bl, 128)
                                    acp = scps.tile([128, 512], f32, tag="ac", name="ac", bufs=2)
                                    nc.tensor.matmul(acp[:], qu_h[:, lsl],
                                                     k_h[:, ts(n, 512)],
                                                     start=True, stop=True)
                                    bdp = scps.tile([128, 640], f32, tag="bd", name="bd", bufs=1)
                                    c0 = 384 - 128 * bl
                                    nc.tensor.matmul(bdp[:, 0:512], qv_h[:, lsl],
                                                     p_h[:, c0:c0 + 512],
                                                     start=True, stop=True)
                                    nc.tensor.matmul(bdp[:, 512:640], qv_h[:, lsl],
                                                     p_h[:, c0 + 512:c0 + 640],
                                                     start=True, stop=True)
                                    band = sc.tile([128, 640], f32, tag="band", name="band", bufs=4)
                                    nc.vector.tensor_copy(band[:], bdp[:])
                                    bds = sc.tile([128, 512], f32, tag="bds", name="bds", bufs=3)
                                    nc.sync.dma_start(
                                        bds[:],
                                        bass.AP(band.tensor, 127, [[639, 128], [1, 512]]))
                                    S = sc.tile([128, 512], f32, tag="S", name="S", bufs=3)
                                    nc.vector.tensor_add(S[:], acp[:], bds[:])
                                    pr = sc.tile([128, 512], mdt, tag="pr", name="pr", bufs=5)
                                    z = sc.tile([128, 1], f32, tag="z", name="z", bufs=4)
                                    nc.scalar.activation(pr[:], S[:], AF.Exp,
                                                         scale=SCALE, accum_out=z[:])
                                    r = sc.tile([128, 1], f32, tag="r", name="r", bufs=4)
                                    nc.vector.reciprocal(r[:], z[:])
                                    prn = sc.tile([128, 512], dt.bfloat16, tag="prn",
                                                  name="prn", bufs=5)
                                    nc.vector.tensor_scalar_mul(prn[:], pr[:], r[:])
                                    probs.append(prn)
                                for sb in range(4):
                                    tp = scps.tile([128, 512], f32, tag="tp", name="tp", bufs=2)
                                    for bl in range(4):
                                        nc.tensor.matmul(tp[:, ts(bl, 128)],
                                                         probs[bl][:, ts(sb, 128)],
                                                         idb[:],
                                                         start=True, stop=True)
                                    pt = sc.tile([128, 512], mdt, tag="pt", name="pt", bufs=4)
                                    (nc.vector.tensor_copy if sb % 2 else
                                     nc.scalar.copy)(pt[:], tp[:])
                                    nc.tensor.matmul(
                                        ao[:],
                                        vT[n * 4 + sb][:, ts(hpair, 128)],
                                        pt[:], start=sb == 0, stop=sb == 3,
                                        skip_group_check=True)
                            for h2 in range(2):
                                b0 = h2 * 64
                                nc.scalar.activation(
                                    att[hpair][b0:b0 + 64, ts(n, 512)],
                                    aos[h2][b0:b0 + 64, :], AF.Identity,
                                    bias=bva[b0:b0 + 64, hpair:hpair + 1])

            if go():  # S6 out_proj
                with tc.tile_pool(name="wo", bufs=1) as wop, \
                     tc.tile_pool(name="wops", bufs=2, space="PSUM") as wps:
                    wo = load_wtiles(wop, dd["woT"], D, "wo")
                    bo = wop.tile([128, DT], f32, tag="bo", name="bo", bufs=1)
                    nc.sync.dma_start(bo[:], dd["bo"])
                    for md in range(DT):
                        for t in range(TC):
                            op_ = wps.tile([128, 512], f32, tag="op", name="op", bufs=2)
                            for kc in range(DT):
                                nc.tensor.matmul(op_[:], wo[kc][:, ts(md, 128)],
                                                 att[kc][:, ts(t, 512)],
                                                 start=kc == 0, stop=kc == DT - 1)
                            nc.vector.scalar_tensor_tensor(
                                x[md][:, ts(t, 512)], op_[:], bo[:, md:md + 1],
                                x[md][:, ts(t, 512)], op0=OP.add, op1=OP.add)

        # ---------------- conv module --------------------------------------
        if go():  # S7 conv
            layer_norm()
            with tc.tile_pool(name="conv", bufs=1) as cp, \
                 tc.tile_pool(name="cvps", bufs=2, space="PSUM") as cps:
                glu_a = [cp.tile([128, T], mdt, tag=f"ga{j}", name=f"ga{j}", bufs=1) for j in range(DT)]
                glu_g = [cp.tile([128, T], f32, tag=f"gg{j}", name=f"gg{j}", bufs=1) for j in range(DT)]
                with tc.tile_pool(name="wc1", bufs=1) as wc1p:
                    wc1 = load_wtiles(wc1p, dd["wc1T"], 2 * D, "wc1")
                    bc1 = wc1p.tile([128, 2 * DT], f32, tag="bc1", name="bc1", bufs=1)
                    nc.sync.dma_start(bc1[:], dd["bc1"])
                    for mf in range(2 * DT):
                        for t in range(TC):
                            cpp = cps.tile([128, 512], f32, tag="c1", name="c1", bufs=2)
                            for kc in range(DT):
                                nc.tensor.matmul(cpp[:], wc1[kc][:, ts(mf, 128)],
                                                 xln[kc][:, ts(t, 512)],
                                                 start=kc == 0, stop=kc == DT - 1)
                            if mf < DT:
                                nc.scalar.activation(glu_a[mf][:, ts(t, 512)], cpp[:],
                                                     AF.Identity, bias=bc1[:, mf:mf + 1])
                            else:
                                nc.scalar.activation(glu_g[mf - DT][:, ts(t, 512)],
                                                     cpp[:], AF.Sigmoid,
                                                     bias=bc1[:, mf:mf + 1])
                wdw = cp.tile([128, DT * KW], f32, tag="wdw", name="wdw", bufs=1)
                nc.sync.dma_start(wdw[:], dd["wdw"])
                bdw = cp.tile([128, DT], f32, tag="bdw", name="bdw", bufs=1)
                nc.sync.dma_start(bdw[:], dd["bdw"])
                # depthwise conv as 31 diagonal-weight matmuls accumulating in
                # PSUM; GLU output lives in a zero-padded buffer so every tap
                # covers the full 512 columns (uniform has_written).
                PW = 2 * PAD + L + 2 * PAD + L + 2 * PAD      # 1084
                glu_pad = [cp.tile([128, PW], mdt, tag=f"gp{j}", name=f"gp{j}",
                                   bufs=1) for j in range(DT)]
                glu_out = [cp.tile([128, T], mdt, tag=f"go{j}", name=f"go{j}",
                                   bufs=1) for j in range(DT)]
                for j in range(DT):
                    nc.vector.memset(glu_pad[j][:].bitcast(f32), 0.0)
                    for nb in range(NB):
                        o = PAD + nb * (L + 2 * PAD)
                        nc.vector.tensor_mul(glu_pad[j][:, o:o + L],
                                             glu_a[j][:, ts(nb, L)],
                                             glu_g[j][:, ts(nb, L)])
                for j in range(DT):
                    ccs = [cps.tile([128, 512], f32, tag="dw", name="dw", bufs=4)
                           for _ in range(NB)]
                    for k in range(KW):
                        dgc = cp.tile([128, 128], mdt, tag="dgc", name="dgc", bufs=4)
                        nc.gpsimd.affine_select(
                            dgc[:], wdw[:, j * KW + k:j * KW + k + 1]
                            .broadcast_to([128, 128]),
                            pattern=[[1, 128]], compare_op=OP.is_equal,
                            fill=0.0, base=0, channel_multiplier=-1)
                        for nb in range(NB):
                            nc.tensor.matmul(ccs[nb][:],
                                             dgc[:],
                                             glu_pad[j][:, nb * (L + 2 * PAD) + k:
                                                        nb * (L + 2 * PAD) + k + 512],
                                             start=k == 0, stop=k == KW - 1,
                                             skip_group_check=True)
                    for nb in range(NB):
                        nc.scalar.activation(glu_out[j][:, ts(nb, 512)], ccs[nb][:],
                                             AF.Silu, bias=bdw[:, j:j + 1])
                with tc.tile_pool(name="wc2", bufs=1) as wc2p:
                    wc2 = load_wtiles(wc2p, dd["wc2T"], D, "wc2")
                    bc2 = wc2p.tile([128, DT], f32, tag="bc2", name="bc2", bufs=1)
                    nc.sync.dma_start(bc2[:], dd["bc2"])
                    for md in range(DT):
                        for t in range(TC):
                            op_ = cps.tile([128, 512], f32, tag="c2", name="c2", bufs=2)
                            for kc in range(DT):
                                nc.tensor.matmul(op_[:], wc2[kc][:, ts(md, 128)],
                                                 glu_out[kc][:, ts(t, 512)],
                                                 start=kc == 0, stop=kc == DT - 1)
                            nc.vector.scalar_tensor_tensor(
                                x[md][:, ts(t, 512)], op_[:], bc2[:, md:md + 1],
                                x[md][:, ts(t, 512)], op0=OP.add, op1=OP.add)

        if go():  # S8 ffn2+final
            ffn("ff")

            # ---------------- final layernorm + output -------------------------
            layer_norm()
            gf = root.tile([128, DT], f32, tag="gf", name="gf", bufs=1)
            bf = root.tile([128, DT], f32, tag="bf", name="bf", bufs=1)
            nc.sync.dma_start(gf[:], dd["gf"])
            nc.sync.dma_start(bf[:], dd["bf"])
            for j in range(DT):
                nc.scalar.activation(xln[j][:], xln[j][:], AF.Identity,
                                     bias=bf[:, j:j + 1], scale=gf[:, j:j + 1])
                nc.sync.dma_start(y_d[ts(j, 128), :], xln[j][:].bitcast(f32))


# ------------------------------------------------------------ host side ---
def _col128(b):
    """[n*128] -> [128, n] column-bias layout."""
    n = b.size // 128
    return np.ascontiguousarray(b.reshape(n, 128).T).astype(np.float32)


def _prep(inputs):
    i = {k: np.asarray(v, np.float32) for k, v in inputs.items()}
    w = {}
    w["posT"] = np.zeros((D, 1024), np.float32)
    w["posT"][:, :2 * L - 1] = i["pos_emb"][0].T

    for mod, pre in (("ffm", "ffm"), ("ff", "ff")):
        g, b = i[f"ln_{pre}_g"], i[f"ln_{pre}_b"]
        w[f"w1T_{mod}"] = np.ascontiguousarray((i[f"{pre}_w1"] * g).T)
        w[f"b1_{mod}"] = _col128(i[f"{pre}_b1"] + i[f"{pre}_w1"] @ b)
        w[f"w2T_{mod}"] = np.ascontiguousarray(i[f"{pre}_w2"].T)
        w[f"b2_{mod}"] = i[f"{pre}_b2"].reshape(1, D).copy()

    g, b = i["ln_mha_g"], i["ln_mha_b"]
    win = i["in_proj_w"] * g
    binp = i["in_proj_b"] + i["in_proj_w"] @ b
    w["winT"] = np.ascontiguousarray(win.T)
    w["bqu"] = _col128(binp[0:D] + i["pos_bias_u"].reshape(D))
    w["bqv"] = _col128(binp[0:D] + i["pos_bias_v"].reshape(D))
    w["bva"] = _col128(binp[2 * D:3 * D])
    w["woT"] = np.ascontiguousarray(i["out_proj_w"].T)
    w["bo"] = _col128(i["out_proj_b"])
    w["pwT"] = np.ascontiguousarray(i["pos_w"].T)

    g, b = i["ln_conv_g"], i["ln_conv_b"]
    w["wc1T"] = np.ascontiguousarray((i["conv_pw1_w"] * g).T)
    w["bc1"] = _col128(i["conv_pw1_b"] + i["conv_pw1_w"] @ b)
    alpha = i["bn_gamma"] / np.sqrt(i["bn_var"] + EPS)
    wdw = i["conv_dw_w"] * alpha[:, None]               # [512, 31]
    w["wdw"] = np.ascontiguousarray(
        wdw.reshape(DT, 128, KW).transpose(1, 0, 2).reshape(128, DT * KW))
    w["bdw"] = _col128((i["conv_dw_b"] - i["bn_mean"]) * alpha + i["bn_beta"])
    w["wc2T"] = np.ascontiguousarray(i["conv_pw2_w"].T)
    w["bc2"] = _col128(i["conv_pw2_b"])
    w["gf"] = _col128(i["ln_final_g"])
    w["bf"] = _col128(i["ln_final_b"])
    return i, w


_NC_CACHE = []


def get_nc():
    if not _NC_CACHE:
        _NC_CACHE.append(_build())
    return _NC_CACHE[0]


def make_in_maps(inputs):
    i, w = _prep(inputs)
    in_maps = []
    for c in range(NCORES):
        m = dict(w)
        # src (L, N, D) -> [D, T] with t = n*L + l
        m["x"] = np.ascontiguousarray(
            i["src"][:, NB * c:NB * (c + 1), :].transpose(2, 1, 0).reshape(D, T))
        in_maps.append(m)
    return in_maps


def assemble(results):
    out = np.empty((L, N, D), np.float32)
    for c in range(NCORES):
        y = results[c]["y"]                             # [D, T]
        for nb in range(NB):
            out[:, NB * c + nb, :] = y[:, nb * L:(nb + 1) * L].T
    return out


def kernel(**inputs):
    nc = get_nc()
    res = run_bass_kernel_spmd(nc, make_in_maps(inputs), list(range(NCORES)))
    return assemble(res.results)



# revision 8
# speedup vs baseline: 9.2382x; 9.2382x over previous
"""Conformer encoder layer on 8 Trainium2 NeuronCores (Bass/Tile).

Sharding: data-parallel over batch N=16 -> 2 batch elements per core, no
collectives. Per-core activations live in channel-major layout [D, T] with
T = n_local*512 + l (each batch's sequence contiguous), which keeps every
matmul in lhsT.T @ rhs form without activation transposes.

Key techniques:
  - LayerNorm gammas and module biases folded into weights host-side;
    mean/E[x^2] computed via (1/D)-matmuls on the PE whose output is
    pre-broadcast to all 128 partitions, applied with DVE tensor ops.
  - rel_shift as one diagonal-AP SBUF->SBUF DMA per score tile:
    out[l', j] = band[l', 127 - l' + j] is affine in flat SBUF space.
  - softmax normalization fused into the score-transpose matmul: the
    transpose of exp-scores is a matmul with diag(1/Z) as moving operand.
  - depthwise conv as 31 shifted multiply-accumulate ops split across the
    vector and gpsimd engines; BatchNorm folded into the conv weights.
"""
from contextlib import ExitStack

import numpy as np

import concourse.bass as bass
import concourse.bacc as bacc
import concourse.tile as tile
import concourse.mybir as mybir
from concourse.bass_utils import run_bass_kernel_spmd

dt = mybir.dt
AF = mybir.ActivationFunctionType
OP = mybir.AluOpType
PM = mybir.MatmulPerfMode.DoubleRow
ts = bass.ts
f32 = dt.float32
f8 = dt.float8e4
F32R = True               # use fp32r (bf16-pair) matmuls: ~4x PE throughput
REPEAT = 1                # duplicate whole program (timing experiments)
STAGES = 99               # truncate program after N stages (timing experiments)
mdt = dt.float32r if F32R else dt.float32
SW = 64.0                 # fp8 weight pre-scale (weights ~0.02 are denormal in e4m3)

D, H, DH, F, KW, L, N = 512, 8, 64, 2048, 31, 512, 16
EPS = 1e-5
NCORES = 8
NB = N // NCORES          # batches per core (2)
T = NB * L                # tokens per core (1024)
DT = D // 128             # channel tiles (4)
FT = F // 128             # FFN hidden tiles (16)
TC = T // 512             # 512-token chunks (2)
PAD = KW // 2             # conv padding (15)
SCALE = DH ** -0.5        # 0.125


def _build():
    nc = bacc.Bacc("TRN2", target_bir_lowering=False, debug=False,
                   num_devices=NCORES)

    def I(name, shape, d=f32):
        return nc.dram_tensor(name, list(shape), d, kind="ExternalInput").ap()

    dd = {
        "x": I("x", (D, T), mdt),
        "posT": I("posT", (D, 1024), mdt),
        "winT": I("winT", (D, 3 * D), mdt),
        "bqu": I("bqu", (128, DT)), "bqv": I("bqv", (128, DT)),
        "bva": I("bva", (128, DT)),
        "woT": I("woT", (D, D), mdt), "bo": I("bo", (128, DT)),
        "pwT": I("pwT", (D, D), mdt),
        "wc1T": I("wc1T", (D, 2 * D), mdt), "bc1": I("bc1", (128, 2 * DT)),
        "wdw": I("wdw", (128, DT * KW)), "bdw": I("bdw", (128, DT)),
        "wc2T": I("wc2T", (D, D), mdt), "bc2": I("bc2", (128, DT)),
        "gf": I("gf", (128, DT)), "bf": I("bf", (128, DT)),
    }
    for m in ("ffm", "ff"):
        dd[f"w1f8_{m}"] = I(f"w1f8_{m}", (128, DT, F), f8)
        dd[f"b1_{m}"] = I(f"b1_{m}", (128, FT))
        dd[f"w2f8_{m}"] = I(f"w2f8_{m}", (128, FT, D), f8)
        dd[f"b2_{m}"] = I(f"b2_{m}", (1, D), mdt)
    y_d = nc.dram_tensor("y", [D, T], f32, kind="ExternalOutput").ap()

    with tile.TileContext(nc) as tc:
        for _rep in range(REPEAT):
            _emit(nc, tc, dd, y_d)
    nc.compile()
    return nc


def _emit(nc, tc, dd, y_d):
    ctx = ExitStack()
    with ctx:
        root = ctx.enter_context(tc.tile_pool(name="root", bufs=1))

        x = [root.tile([128, T], mdt, tag=f"x{j}", name=f"x{j}", bufs=1) for j in range(DT)]
        for j in range(DT):
            nc.sync.dma_start(x[j][:], dd["x"][ts(j, 128), :])
        ones = root.tile([128, 128], mdt, tag="ones", name="ones", bufs=1)
        onesrow = root.tile([1, 512], mdt, tag="onesrow", name="onesrow", bufs=1)
        cst = root.tile([128, 512], f32, tag="cst", name="cst", bufs=1)
        nc.vector.memset(cst[:], 1.0 / D)
        nc.scalar.copy(ones[:], cst[:, :128])
        nc.vector.memset(cst[:1, :], 1.0)
        nc.scalar.copy(onesrow[:], cst[:1, :])
        idb = root.tile([128, 128], dt.bfloat16, tag="idb", name="idb", bufs=1)
        nc.vector.memset(cst[:, :1], 1.0)
        nc.gpsimd.affine_select(idb[:], cst[:, 0:1].broadcast_to([128, 128]),
                                pattern=[[1, 128]], compare_op=OP.is_equal,
                                fill=0.0, base=0, channel_multiplier=-1)
        xln = [root.tile([128, T], mdt, tag=f"xln{j}", name=f"xln{j}", bufs=1) for j in range(DT)]
        xln8 = root.tile([128, DT, T], f8, tag="xln8", name="xln8", bufs=1)
        s_b = root.tile([128, T], mdt, tag="s_b", name="s_b", bufs=1)
        ms_b = root.tile([128, T], mdt, tag="ms_b", name="ms_b", bufs=1)

        # ---------------- layernorm: stats + apply -> xln ------------------
        def layer_norm(fp8=False):
            with tc.tile_pool(name="lnp", bufs=1, space="PSUM") as lnps, \
                 tc.tile_pool(name="lns", bufs=2) as lns:
                mp = lnps.tile([128, T], f32, tag="m", name="m", bufs=1)
                qp = lnps.tile([128, T], f32, tag="q", name="q", bufs=1)
                for kc in range(DT):
                    x2 = lns.tile([128, T], mdt, tag="x2", name="x2", bufs=2)
                    nc.scalar.square(x2[:], x[kc][:])
                    for t in range(TC):
                        nc.tensor.matmul(mp[:, ts(t, 512)], ones[:],
                                         x[kc][:, ts(t, 512)],
                                         start=kc == 0, stop=kc == DT - 1,
                                         skip_group_check=True)
                        nc.tensor.matmul(qp[:, ts(t, 512)], ones[:],
                                         x2[:, ts(t, 512)],
                                         start=kc == 0, stop=kc == DT - 1,
                                         skip_group_check=True)
                msq = lns.tile([128, T], f32, tag="tmp", name="tmp", bufs=3)
                nc.scalar.square(msq[:], mp[:])
                veps = lns.tile([128, T], f32, tag="tmp", name="tmp", bufs=3)
                nc.vector.scalar_tensor_tensor(veps[:], qp[:], EPS, msq[:],
                                               op0=OP.add, op1=OP.subtract)
                rec = lns.tile([128, T], f32, tag="tmp", name="tmp", bufs=3)
                nc.vector.reciprocal(rec[:], veps[:])
                nc.scalar.sqrt(s_b[:], rec[:])
                nc.vector.tensor_mul(ms_b[:], mp[:], s_b[:])
                for kc in range(DT):
                    u = lns.tile([128, T], f32, tag="tmp", name="tmp", bufs=3)
                    eng = nc.vector if kc < 2 else nc.gpsimd
                    eng.tensor_mul(u[:], x[kc][:], s_b[:])
                    dst = xln8[:, kc, :] if fp8 else xln[kc][:]
                    eng.tensor_sub(dst, u[:], ms_b[:])

        def load_wtiles(pool, w_d, ncols, tag):
            w = [pool.tile([128, ncols], mdt, tag=f"{tag}{j}", name=f"{tag}{j}", bufs=1)
                 for j in range(DT)]
            for j in range(DT):
                nc.sync.dma_start(w[j][:], w_d[ts(j, 128), :])
            return w

        # ---------------- FFN (fp8 DoubleRow matmuls) ----------------------
        def ffn(mod):
            layer_norm(fp8=True)
            with tc.tile_pool(name=f"h_{mod}", bufs=1) as hpool, \
                 tc.tile_pool(name=f"ps_{mod}", bufs=1, space="PSUM") as ps:
                h8 = hpool.tile([128, FT, T], f8, tag="h8", name="h8", bufs=1)
                with tc.tile_pool(name=f"w1_{mod}", bufs=1) as w1p:
                    w1 = w1p.tile([128, DT, F], f8, tag="w1", name="w1", bufs=1)
                    nc.sync.dma_start(w1[:], dd[f"w1f8_{mod}"])
                    b1 = w1p.tile([128, FT], f32, tag="b1", name="b1", bufs=1)
                    nc.sync.dma_start(b1[:], dd[f"b1_{mod}"])
                    for mf in range(FT):
                        for t in range(TC):
                            hp = ps.tile([128, 512], f32, tag="hp", name="hp", bufs=2)
                            for p in range(DT // 2):
                                nc.tensor.matmul(hp[:],
                                                 w1[:, 2 * p:2 * p + 2, ts(mf, 128)],
                                                 xln8[:, 2 * p:2 * p + 2, ts(t, 512)],
                                                 start=p == 0, stop=p == DT // 2 - 1,
                                                 perf_mode=PM)
                            nc.scalar.activation(h8[:, mf, ts(t, 512)], hp[:],
                                                 AF.Silu, scale=1.0 / SW,
                                                 bias=b1[:, mf:mf + 1])
                with tc.tile_pool(name=f"w2_{mod}", bufs=1) as w2p:
                    w2 = w2p.tile([128, FT, D], f8, tag="w2", name="w2", bufs=1)
                    nc.sync.dma_start(w2[:], dd[f"w2f8_{mod}"])
                    b2 = w2p.tile([1, D], mdt, tag="b2", name="b2", bufs=1)
                    nc.sync.dma_start(b2[:], dd[f"b2_{mod}"])
                    for md in range(DT):
                        yp = [ps.tile([128, 512], f32, tag="yp", name="yp", bufs=2)
                              for _ in range(TC)]
                        for q in range(FT // 2):
                            for t in range(TC):
                                nc.tensor.matmul(yp[t][:],
                                                 w2[:, 2 * q:2 * q + 2, ts(md, 128)],
                                                 h8[:, 2 * q:2 * q + 2, ts(t, 512)],
                                                 start=q == 0, stop=False,
                                                 perf_mode=PM,
                                                 skip_group_check=True)
                        for t in range(TC):
                            nc.tensor.matmul(yp[t][:], b2[:, ts(md, 128)],
                                             onesrow[:], start=False, stop=True,
                                             skip_group_check=True)
                            nc.vector.scalar_tensor_tensor(
                                x[md][:, ts(t, 512)], yp[t][:], 0.5 / SW,
                                x[md][:, ts(t, 512)], op0=OP.mult, op1=OP.add)

        # =========================== program ===============================
        stage = [0]
        def go():
            stage[0] += 1
            return STAGES >= stage[0]

        p = [root.tile([128, 1024], mdt, tag=f"p{j}", name=f"p{j}", bufs=1) for j in range(DT)]
        if go():  # S1 pos
            with tc.tile_pool(name="pos", bufs=1) as pospool, \
                 tc.tile_pool(name="posps", bufs=2, space="PSUM") as posps:
                posT = load_wtiles(pospool, dd["posT"], 1024, "posT")
                pwT = load_wtiles(pospool, dd["pwT"], D, "pwT")
                for mf in range(DT):
                    for t in range(2):
                        pp = posps.tile([128, 512], f32, tag="pp", name="pp", bufs=2)
                        for kc in range(DT):
                            nc.tensor.matmul(pp[:], pwT[kc][:, ts(mf, 128)],
                                             posT[kc][:, ts(t, 512)],
                                             start=kc == 0, stop=kc == DT - 1)
                        nc.scalar.copy(p[mf][:, ts(t, 512)], pp[:])

        if go():  # S2 ffn1
            ffn("ffm")

        # ---------------- attention ---------------------------------------
        layer_norm()
        with tc.tile_pool(name="attn", bufs=1) as ap:
            q_u = [ap.tile([128, T], mdt, tag=f"qu{j}", name=f"qu{j}", bufs=1) for j in range(DT)]
            q_v = [ap.tile([128, T], mdt, tag=f"qv{j}", name=f"qv{j}", bufs=1) for j in range(DT)]
            ksb = [ap.tile([128, T], mdt, tag=f"k{j}", name=f"k{j}", bufs=1) for j in range(DT)]
            vT = [ap.tile([128, D], mdt, tag=f"vT{t}", name=f"vT{t}", bufs=1) for t in range(8)]
            att = [ap.tile([128, T], mdt, tag=f"att{j}", name=f"att{j}", bufs=1) for j in range(DT)]
            bqu = ap.tile([128, DT], f32, tag="bqu", name="bqu", bufs=1)
            bqv = ap.tile([128, DT], f32, tag="bqv", name="bqv", bufs=1)
            bva = ap.tile([128, DT], f32, tag="bva", name="bva", bufs=1)
            for t_, nm in ((bqu, "bqu"), (bqv, "bqv"), (bva, "bva")):
                nc.sync.dma_start(t_[:], dd[nm])

            if go():  # S4 qkv
                with tc.tile_pool(name="win", bufs=1) as wp, \
                     tc.tile_pool(name="qkvps", bufs=2, space="PSUM") as qps:
                    win = load_wtiles(wp, dd["winT"], 3 * D, "win")
                    for mf in range(2 * DT):          # q then k channel tiles
                        for t in range(TC):
                            qp = qps.tile([128, 512], f32, tag="qp", name="qp", bufs=2)
                            for kc in range(DT):
                                nc.tensor.matmul(qp[:], win[kc][:, ts(mf, 128)],
                                                 xln[kc][:, ts(t, 512)],
                                                 start=kc == 0, stop=kc == DT - 1)
                            if mf < DT:
                                nc.scalar.activation(q_u[mf][:, ts(t, 512)], qp[:],
                                                     AF.Identity, bias=bqu[:, mf:mf + 1])
                                nc.scalar.activation(q_v[mf][:, ts(t, 512)], qp[:],
                                                     AF.Identity, bias=bqv[:, mf:mf + 1])
                            else:
                                nc.scalar.copy(ksb[mf - DT][:, ts(t, 512)], qp[:])
                    for tt in range(8):               # v, transposed layout
                        vp = qps.tile([128, 512], f32, tag="qp", name="qp", bufs=2)
                        for kc in range(DT):
                            nc.tensor.matmul(vp[:], xln[kc][:, ts(tt, 128)],
                                             win[kc][:, 2 * D:3 * D],
                                             start=kc == 0, stop=kc == DT - 1)
                        nc.scalar.copy(vT[tt][:], vp[:])

            if go():  # S5 scores
                with tc.tile_pool(name="sc", bufs=1) as sc, \
                     tc.tile_pool(name="scps", bufs=1, space="PSUM") as scps:
                    for n in range(NB):
                        for hpair in range(H // 2):
                            aos = [scps.tile([128, 512], f32, tag="ao", name="ao",
                                             bufs=2) for _ in range(2)]
                            for h2 in range(2):
                                ao = aos[h2]
                                h_ = 2 * hpair + h2
                                mt, base = h_ // 2, (h_ % 2) * 64
                                qu_h = q_u[mt][base:base + 64, :]
                                qv_h = q_v[mt][base:base + 64, :]
                                k_h = ksb[mt][base:base + 64, :]
                                p_h = p[mt][base:base + 64, :]
                                probs = []
                                for bl in range(4):
                                    lsl = ts(n * 4 + bl, 128)
                                    acp = scps.tile([128, 512], f32, tag="ac", name="ac", bufs=2)
                                    nc.tensor.matmul(acp[:], qu_h[:, lsl],
                                                     k_h[:, ts(n, 512)],
                                                     start=True, stop=True)
                                    bdp = scps.tile([128, 640], f32, tag="bd", name="bd", bufs=1)
                                    c0 = 384 - 128 * bl
                                    nc.tensor.matmul(bdp[:, 0:512], qv_h[:, lsl],
                                                     p_h[:, c0:c0 + 512],
                                                     start=True, stop=True)
                                    nc.tensor.matmul(bdp[:, 512:640], qv_h[:, lsl],
                                                     p_h[:, c0 + 512:c0 + 640],
                                                     start=True, stop=True)
                                    band = sc.tile([128, 640], f32, tag="band", name="band", bufs=4)
                                    nc.vector.tensor_copy(band[:], bdp[:])
                                    bds = sc.tile([128, 512], f32, tag="bds", name="bds", bufs=3)
                                    nc.sync.dma_start(
                                        bds[:],
                                        bass.AP(band.tensor, 127, [[639, 128], [1, 512]]))
                                    S = sc.tile([128, 512], f32, tag="S", name="S", bufs=3)
                                    nc.vector.tensor_add(S[:], acp[:], bds[:])
                                    pr = sc.tile([128, 512], mdt, tag="pr", name="pr", bufs=5)
                                    z = sc.tile([128, 1], f32, tag="z", name="z", bufs=4)
                                    nc.scalar.activation(pr[:], S[:], AF.Exp,
                                                         scale=SCALE, accum_out=z[:])
                                    r = sc.tile([128, 1], f32, tag="r", name="r", bufs=4)
                                    nc.vector.reciprocal(r[:], z[:])
                                    prn = sc.tile([128, 512], dt.bfloat16, tag="prn",
                                                  name="prn", bufs=5)
                                    nc.vector.tensor_scalar_mul(prn[:], pr[:], r[:])
                                    probs.append(prn)
                                for sb in range(4):
                                    tp = scps.tile([128, 512], f32, tag="tp", name="tp", bufs=2)
                                    for bl in range(4):
                                        nc.tensor.matmul(tp[:, ts(bl, 128)],
                                                         probs[bl][:, ts(sb, 128)],
                                                         idb[:],
                                                         start=True, stop=True)
                                    pt = sc.tile([128, 512], mdt, tag="pt", name="pt", bufs=4)
                                    (nc.vector.tensor_copy if sb % 2 else
                                     nc.scalar.copy)(pt[:], tp[:])
                                    nc.tensor.matmul(
                                        ao[:],
                                        vT[n * 4 + sb][:, ts(hpair, 128)],
                                        pt[:], start=sb == 0, stop=sb == 3,
                                        skip_group_check=True)
                            for h2 in range(2):
                                b0 = h2 * 64
                                nc.scalar.activation(
                                    att[hpair][b0:b0 + 64, ts(n, 512)],
                                    aos[h2][b0:b0 + 64, :], AF.Identity,
                                    bias=bva[b0:b0 + 64, hpair:hpair + 1])

            if go():  # S6 out_proj
                with tc.tile_pool(name="wo", bufs=1) as wop, \
                     tc.tile_pool(name="wops", bufs=2, space="PSUM") as wps:
                    wo = load_wtiles(wop, dd["woT"], D, "wo")
                    bo = wop.tile([128, DT], f32, tag="bo", name="bo", bufs=1)
                    nc.sync.dma_start(bo[:], dd["bo"])
                    for md in range(DT):
                        for t in range(TC):
                            op_ = wps.tile([128, 512], f32, tag="op", name="op", bufs=2)
                            for kc in range(DT):
                                nc.tensor.matmul(op_[:], wo[kc][:, ts(md, 128)],
                                                 att[kc][:, ts(t, 512)],
                                                 start=kc == 0, stop=kc == DT - 1)
                            nc.vector.scalar_tensor_tensor(
                                x[md][:, ts(t, 512)], op_[:], bo[:, md:md + 1],
                                x[md][:, ts(t, 512)], op0=OP.add, op1=OP.add)

        # ---------------- conv module --------------------------------------
        if go():  # S7 conv
            layer_norm()
            with tc.tile_pool(name="conv", bufs=1) as cp, \
                 tc.tile_pool(name="cvps", bufs=2, space="PSUM") as cps:
                glu_a = [cp.tile([128, T], mdt, tag=f"ga{j}", name=f"ga{j}", bufs=1) for j in range(DT)]
                glu_g = [cp.tile([128, T], f32, tag=f"gg{j}", name=f"gg{j}", bufs=1) for j in range(DT)]
                with tc.tile_pool(name="wc1", bufs=1) as wc1p:
                    wc1 = load_wtiles(wc1p, dd["wc1T"], 2 * D, "wc1")
                    bc1 = wc1p.tile([128, 2 * DT], f32, tag="bc1", name="bc1", bufs=1)
                    nc.sync.dma_start(bc1[:], dd["bc1"])
                    for mf in range(2 * DT):
                        for t in range(TC):
                            cpp = cps.tile([128, 512], f32, tag="c1", name="c1", bufs=2)
                            for kc in range(DT):
                                nc.tensor.matmul(cpp[:], wc1[kc][:, ts(mf, 128)],
                                                 xln[kc][:, ts(t, 512)],
                                                 start=kc == 0, stop=kc == DT - 1)
                            if mf < DT:
                                nc.scalar.activation(glu_a[mf][:, ts(t, 512)], cpp[:],
                                                     AF.Identity, bias=bc1[:, mf:mf + 1])
                            else:
                                nc.scalar.activation(glu_g[mf - DT][:, ts(t, 512)],
                                                     cpp[:], AF.Sigmoid,
                                                     bias=bc1[:, mf:mf + 1])
                wdw = cp.tile([128, DT * KW], f32, tag="wdw", name="wdw", bufs=1)
                nc.sync.dma_start(wdw[:], dd["wdw"])
                bdw = cp.tile([128, DT], f32, tag="bdw", name="bdw", bufs=1)
                nc.sync.dma_start(bdw[:], dd["bdw"])
                # depthwise conv as 31 diagonal-weight matmuls accumulating in
                # PSUM; GLU output lives in a zero-padded buffer so every tap
                # covers the full 512 columns (uniform has_written).
                PW = 2 * PAD + L + 2 * PAD + L + 2 * PAD      # 1084
                glu_pad = [cp.tile([128, PW], mdt, tag=f"gp{j}", name=f"gp{j}",
                                   bufs=1) for j in range(DT)]
                glu_out = [cp.tile([128, T], mdt, tag=f"go{j}", name=f"go{j}",
                                   bufs=1) for j in range(DT)]
                for j in range(DT):
                    nc.vector.memset(glu_pad[j][:].bitcast(f32), 0.0)
                    for nb in range(NB):
                        o = PAD + nb * (L + 2 * PAD)
                        nc.vector.tensor_mul(glu_pad[j][:, o:o + L],
                                             glu_a[j][:, ts(nb, L)],
                                             glu_g[j][:, ts(nb, L)])
                for j in range(DT):
                    ccs = [cps.tile([128, 512], f32, tag="dw", name="dw", bufs=4)
                           for _ in range(NB)]
                    for k in range(KW):
                        dgc = cp.tile([128, 128], mdt, tag="dgc", name="dgc", bufs=4)
                        nc.gpsimd.affine_select(
                            dgc[:], wdw[:, j * KW + k:j * KW + k + 1]
                            .broadcast_to([128, 128]),
                            pattern=[[1, 128]], compare_op=OP.is_equal,
                            fill=0.0, base=0, channel_multiplier=-1)
                        for nb in range(NB):
                            nc.tensor.matmul(ccs[nb][:],
                                             dgc[:],
                                             glu_pad[j][:, nb * (L + 2 * PAD) + k:
                                                        nb * (L + 2 * PAD) + k + 512],
                                             start=k == 0, stop=k == KW - 1,
                                             skip_group_check=True)
                    for nb in range(NB):
                        nc.scalar.activation(glu_out[j][:, ts(nb, 512)], ccs[nb][:],
                                             AF.Silu, bias=bdw[:, j:j + 1])
                with tc.tile_pool(name="wc2", bufs=1) as wc2p:
                    wc2 = load_wtiles(wc2p, dd["wc2T"], D, "wc2")
                    bc2 = wc2p.tile([128, DT], f32, tag="bc2", name="bc2", bufs=1)
                    nc.sync.dma_start(bc2[:], dd["bc2"])
                    for md in range(DT):
                        for t in range(TC):
                            op_ = cps.tile([128, 512], f32, tag="c2", name="c2", bufs=2)
                            for kc in range(DT):
                                nc.tensor.matmul(op_[:], wc2[kc][:, ts(md, 128)],
                                                 glu_out[kc][:, ts(t, 512)],
                                                 start=kc == 0, stop=kc == DT - 1)
                            nc.vector.scalar_tensor_tensor(
                                x[md][:, ts(t, 512)], op_[:], bc2[:, md:md + 1],
                                x[md][:, ts(t, 512)], op0=OP.add, op1=OP.add)

        if go():  # S8 ffn2+final
            ffn("ff")

            # ---------------- final layernorm + output -------------------------
            layer_norm()
            gf = root.tile([128, DT], f32, tag="gf", name="gf", bufs=1)
            bf = root.tile([128, DT], f32, tag="bf", name="bf", bufs=1)
            nc.sync.dma_start(gf[:], dd["gf"])
            nc.sync.dma_start(bf[:], dd["bf"])
            for j in range(DT):
                nc.scalar.activation(xln[j][:], xln[j][:], AF.Identity,
                                     bias=bf[:, j:j + 1], scale=gf[:, j:j + 1])
                nc.sync.dma_start(y_d[ts(j, 128), :], xln[j][:].bitcast(f32))


# ------------------------------------------------------------ host side ---
def _col128(b):
    """[n*128] -> [128, n] column-bias layout."""
    n = b.size // 128
    return np.ascontiguousarray(b.reshape(n, 128).T).astype(np.float32)


def _f8ktiles(wT):
    """[K, M] f32 -> [128, K//128, M] fp8e4 (k-subtile-interleaved, as uint8)."""
    import ml_dtypes
    K, M = wT.shape
    a = wT.reshape(K // 128, 128, M).transpose(1, 0, 2)
    return np.ascontiguousarray(a).astype(ml_dtypes.float8_e4m3).view(np.uint8)


def _prep(inputs):
    i = {k: np.asarray(v, np.float32) for k, v in inputs.items()}
    w = {}
    w["posT"] = np.zeros((D, 1024), np.float32)
    w["posT"][:, :2 * L - 1] = i["pos_emb"][0].T

    for mod, pre in (("ffm", "ffm"), ("ff", "ff")):
        g, b = i[f"ln_{pre}_g"], i[f"ln_{pre}_b"]
        w[f"w1f8_{mod}"] = _f8ktiles((i[f"{pre}_w1"] * g).T * SW)
        w[f"b1_{mod}"] = _col128(i[f"{pre}_b1"] + i[f"{pre}_w1"] @ b)
        w[f"w2f8_{mod}"] = _f8ktiles(i[f"{pre}_w2"].T * SW)
        w[f"b2_{mod}"] = (i[f"{pre}_b2"] * SW).reshape(1, D).copy()

    g, b = i["ln_mha_g"], i["ln_mha_b"]
    win = i["in_proj_w"] * g
    binp = i["in_proj_b"] + i["in_proj_w"] @ b
    w["winT"] = np.ascontiguousarray(win.T)
    w["bqu"] = _col128(binp[0:D] + i["pos_bias_u"].reshape(D))
    w["bqv"] = _col128(binp[0:D] + i["pos_bias_v"].reshape(D))
    w["bva"] = _col128(binp[2 * D:3 * D])
    w["woT"] = np.ascontiguousarray(i["out_proj_w"].T)
    w["bo"] = _col128(i["out_proj_b"])
    w["pwT"] = np.ascontiguousarray(i["pos_w"].T)

    g, b = i["ln_conv_g"], i["ln_conv_b"]
    w["wc1T"] = np.ascontiguousarray((i["conv_pw1_w"] * g).T)
    w["bc1"] = _col128(i["conv_pw1_b"] + i["conv_pw1_w"] @ b)
    alpha = i["bn_gamma"] / np.sqrt(i["bn_var"] + EPS)
    wdw = i["conv_dw_w"] * alpha[:, None]               # [512, 31]
    w["wdw"] = np.ascontiguousarray(
        wdw.reshape(DT, 128, KW).transpose(1, 0, 2).reshape(128, DT * KW))
    w["bdw"] = _col128((i["conv_dw_b"] - i["bn_mean"]) * alpha + i["bn_beta"])
    w["wc2T"] = np.ascontiguousarray(i["conv_pw2_w"].T)
    w["bc2"] = _col128(i["conv_pw2_b"])
    w["gf"] = _col128(i["ln_final_g"])
    w["bf"] = _col128(i["ln_final_b"])
    return i, w


_NC_CACHE = []


def get_nc():
    if not _NC_CACHE:
        _NC_CACHE.append(_build())
    return _NC_CACHE[0]


def make_in_maps(inputs):
    i, w = _prep(inputs)
    in_maps = []
    for c in range(NCORES):
        m = dict(w)
        # src (L, N, D) -> [D, T] with t = n*L + l
        m["x"] = np.ascontiguousarray(
            i["src"][:, NB * c:NB * (c + 1), :].transpose(2, 1, 0).reshape(D, T))
        in_maps.append(m)
    return in_maps


def assemble(results):
    out = np.empty((L, N, D), np.float32)
    for c in range(NCORES):
        y = results[c]["y"]                             # [D, T]
        for nb in range(NB):
            out[:, NB * c + nb, :] = y[:, nb * L:(nb + 1) * L].T
    return out


def kernel(**inputs):
    nc = get_nc()
    res = run_bass_kernel_spmd(nc, make_in_maps(inputs), list(range(NCORES)))
    return assemble(res.results)



# revision 12
# speedup vs baseline: 10.9974x; 1.1904x over previous
"""Conformer encoder layer on 8 Trainium2 NeuronCores (Bass/Tile).

Sharding: data-parallel over batch N=16 -> 2 batch elements per core, no
collectives. Per-core activations live in channel-major layout [D, T] with
T = n_local*512 + l (each batch's sequence contiguous), which keeps every
matmul in lhsT.T @ rhs form without activation transposes.

Key techniques:
  - All GEMM weights quantized host-side to fp8e4m3 (x64 pre-scale to escape
    e4m3 denormals) and every contraction-over-channels matmul runs in
    DoubleRow perf mode: [128,2,M] x [128,2,N] fp8 pairs, 2x PE throughput.
    Descales fold into the Act/vector ops that drain PSUM.
  - All weights resident in SBUF, DMA'd once at program start (no mid-program
    weight-load stalls).
  - pos_emb @ pos_w.T computed host-side; scores path runs in bf16.
  - LayerNorm gammas and module biases folded into weights host-side;
    mean/E[x^2] computed via (1/D)-matmuls on the PE whose output is
    pre-broadcast to all 128 partitions, applied with DVE tensor ops.
  - rel_shift as one diagonal-AP SBUF->SBUF DMA per score tile.
  - depthwise conv as diagonal-weight matmuls (host-precomputed fp8 diagonal
    planes, taps paired via DoubleRow) accumulating in PSUM; BatchNorm folded
    into the conv weights.
"""
from contextlib import ExitStack

import numpy as np

import concourse.bass as bass
import concourse.bacc as bacc
import concourse.tile as tile
import concourse.mybir as mybir
from concourse.bass_utils import run_bass_kernel_spmd

dt = mybir.dt
AF = mybir.ActivationFunctionType
OP = mybir.AluOpType
PM = mybir.MatmulPerfMode.DoubleRow
ts = bass.ts
f32 = dt.float32
f8 = dt.float8e4
bf16 = dt.bfloat16
REPEAT = 1                # duplicate whole program (timing experiments)
STAGES = 99               # truncate program after N stages (timing experiments)
mdt = dt.float32r
SW = 64.0                 # fp8 weight pre-scale (weights ~0.02 are denormal in e4m3)
GS = 8.0                  # glu activation pre-scale

D, H, DH, F, KW, L, N = 512, 8, 64, 2048, 31, 512, 16
EPS = 1e-5
NCORES = 8
NB = N // NCORES          # batches per core (2)
T = NB * L                # tokens per core (1024)
DT = D // 128             # channel tiles (4)
FT = F // 128             # FFN hidden tiles (16)
TC = T // 512             # 512-token chunks (2)
PAD = KW // 2             # conv padding (15)
SCALE = DH ** -0.5        # 0.125
SEG = L + 2 * PAD         # padded per-batch conv segment (542)
PW = 2 * PAD + NB * SEG + 2         # conv pad buffer width (1116, /4 aligned)


def _build():
    nc = bacc.Bacc("TRN2", target_bir_lowering=False, debug=False,
                   num_devices=NCORES)

    def I(name, shape, d=f32):
        return nc.dram_tensor(name, list(shape), d, kind="ExternalInput").ap()

    dd = {
        "x": I("x", (D, T), mdt),
        "pT": I("pT", (D, 1024), bf16),
        "win8": I("win8", (128, DT, 3 * D), f8),
        "bqu": I("bqu", (128, DT)), "bqv": I("bqv", (128, DT)),
        "bva": I("bva", (128, DT)),
        "wo8": I("wo8", (128, DT, D), f8), "bo": I("bo", (1, D), mdt),
        "wc18": I("wc18", (128, DT, 2 * D), f8), "bc1": I("bc1", (128, 2 * DT)),
        "dgc8": I("dgc8", (128, DT, KW, 128), f8), "bdw": I("bdw", (128, DT)),
        "wc28": I("wc28", (128, DT, D), f8), "bc2": I("bc2", (1, D), mdt),
        "gf": I("gf", (128, DT)), "bf": I("bf", (128, DT)),
    }
    for m in ("ffm", "ff"):
        dd[f"w1f8_{m}"] = I(f"w1f8_{m}", (128, DT, F), f8)
        dd[f"b1_{m}"] = I(f"b1_{m}", (128, FT))
        dd[f"w2f8_{m}"] = I(f"w2f8_{m}", (128, FT, D), f8)
        dd[f"b2_{m}"] = I(f"b2_{m}", (1, D), mdt)
    y_d = nc.dram_tensor("y", [D, T], f32, kind="ExternalOutput").ap()

    with tile.TileContext(nc) as tc:
        for _rep in range(REPEAT):
            _emit(nc, tc, dd, y_d)
    nc.compile()
    return nc


def _emit(nc, tc, dd, y_d):
    ctx = ExitStack()
    with ctx:
        root = ctx.enter_context(tc.tile_pool(name="root", bufs=1))

        x = [root.tile([128, T], mdt, tag=f"x{j}", name=f"x{j}", bufs=1) for j in range(DT)]
        for j in range(DT):
            nc.sync.dma_start(x[j][:], dd["x"][ts(j, 128), :])
        ones = root.tile([128, 128], mdt, tag="ones", name="ones", bufs=1)
        onesrow = root.tile([1, 512], mdt, tag="onesrow", name="onesrow", bufs=1)
        cst = root.tile([128, 512], f32, tag="cst", name="cst", bufs=1)
        nc.vector.memset(cst[:], 1.0 / D)
        nc.scalar.copy(ones[:], cst[:, :128])
        nc.vector.memset(cst[:1, :], 1.0)
        nc.scalar.copy(onesrow[:], cst[:1, :])
        idb = root.tile([128, 128], bf16, tag="idb", name="idb", bufs=1)
        nc.vector.memset(cst[:, :1], 1.0)
        nc.gpsimd.affine_select(idb[:], cst[:, 0:1].broadcast_to([128, 128]),
                                pattern=[[1, 128]], compare_op=OP.is_equal,
                                fill=0.0, base=0, channel_multiplier=-1)

        # -------- resident weights: single up-front DMA burst --------------
        wt = {}
        for nm, shape, d_ in (
                ("win8", (128, DT, 3 * D), f8),
                ("bqu", (128, DT), f32), ("bqv", (128, DT), f32),
                ("bva", (128, DT), f32),
                ("wo8", (128, DT, D), f8), ("bo", (1, D), mdt),
                ("wc18", (128, DT, 2 * D), f8), ("bc1", (128, 2 * DT), f32),
                ("dgc8", (128, DT, KW, 128), f8), ("bdw", (128, DT), f32),
                ("wc28", (128, DT, D), f8), ("bc2", (1, D), mdt),
                ("gf", (128, DT), f32), ("bf", (128, DT), f32),
                ("w1f8_ffm", (128, DT, F), f8), ("b1_ffm", (128, FT), f32),
                ("w2f8_ffm", (128, FT, D), f8), ("b2_ffm", (1, D), mdt),
                ("w1f8_ff", (128, DT, F), f8), ("b1_ff", (128, FT), f32),
                ("w2f8_ff", (128, FT, D), f8), ("b2_ff", (1, D), mdt)):
            wt[nm] = root.tile(list(shape), d_, tag=nm, name=nm, bufs=1)
            nc.sync.dma_start(wt[nm][:], dd[nm])
        pT = [root.tile([128, 1024], bf16, tag=f"pT{j}", name=f"pT{j}", bufs=1)
              for j in range(DT)]
        for j in range(DT):
            nc.sync.dma_start(pT[j][:], dd["pT"][ts(j, 128), :])

        xln8 = root.tile([128, DT, T], f8, tag="xln8", name="xln8", bufs=1)
        s_b = root.tile([128, T], mdt, tag="s_b", name="s_b", bufs=1)
        ms_b = root.tile([128, T], mdt, tag="ms_b", name="ms_b", bufs=1)

        # ---------------- layernorm: stats + apply -> xln8 -----------------
        def layer_norm(outs=None):
            with tc.tile_pool(name="lnp", bufs=1, space="PSUM") as lnps, \
                 tc.tile_pool(name="lns", bufs=2) as lns:
                mp = lnps.tile([128, T], f32, tag="m", name="m", bufs=1)
                qp = lnps.tile([128, T], f32, tag="q", name="q", bufs=1)
                for kc in range(DT):
                    x2 = lns.tile([128, T], mdt, tag="x2", name="x2", bufs=2)
                    nc.scalar.square(x2[:], x[kc][:])
                    for t in range(TC):
                        nc.tensor.matmul(mp[:, ts(t, 512)], ones[:],
                                         x[kc][:, ts(t, 512)],
                                         start=kc == 0, stop=kc == DT - 1,
                                         skip_group_check=True)
                        nc.tensor.matmul(qp[:, ts(t, 512)], ones[:],
                                         x2[:, ts(t, 512)],
                                         start=kc == 0, stop=kc == DT - 1,
                                         skip_group_check=True)
                msq = lns.tile([128, T], f32, tag="tmp", name="tmp", bufs=2)
                nc.scalar.square(msq[:], mp[:])
                veps = lns.tile([128, T], f32, tag="tmp", name="tmp", bufs=2)
                nc.vector.scalar_tensor_tensor(veps[:], qp[:], EPS, msq[:],
                                               op0=OP.add, op1=OP.subtract)
                rec = lns.tile([128, T], f32, tag="tmp", name="tmp", bufs=2)
                nc.vector.reciprocal(rec[:], veps[:])
                nc.scalar.sqrt(s_b[:], rec[:])
                nc.vector.tensor_mul(ms_b[:], mp[:], s_b[:])
                for kc in range(DT):
                    u = lns.tile([128, T], f32, tag="tmp", name="tmp", bufs=2)
                    eng = nc.vector if kc < 2 else nc.gpsimd
                    eng.tensor_mul(u[:], x[kc][:], s_b[:])
                    dst = outs[kc] if outs is not None else xln8[:, kc, :]
                    eng.tensor_sub(dst, u[:], ms_b[:])

        # ---------------- FFN (fp8 DoubleRow matmuls) ----------------------
        def ffn(mod):
            layer_norm()
            w1, b1 = wt[f"w1f8_{mod}"], wt[f"b1_{mod}"]
            w2, b2 = wt[f"w2f8_{mod}"], wt[f"b2_{mod}"]
            with tc.tile_pool(name=f"h_{mod}", bufs=1) as hpool, \
                 tc.tile_pool(name=f"ps_{mod}", bufs=1, space="PSUM") as ps:
                h8 = hpool.tile([128, FT, T], f8, tag="h8", name="h8", bufs=1)
                for mf in range(FT):
                    for t in range(TC):
                        hp = ps.tile([128, 512], f32, tag="hp", name="hp", bufs=2)
                        for p in range(DT // 2):
                            nc.tensor.matmul(hp[:],
                                             w1[:, 2 * p:2 * p + 2, ts(mf, 128)],
                                             xln8[:, 2 * p:2 * p + 2, ts(t, 512)],
                                             start=p == 0, stop=p == DT // 2 - 1,
                                             perf_mode=PM)
                        nc.scalar.activation(h8[:, mf, ts(t, 512)], hp[:],
                                             AF.Silu, scale=1.0 / SW,
                                             bias=b1[:, mf:mf + 1])
                for md in range(DT):
                    yp = [ps.tile([128, 512], f32, tag="yp", name="yp", bufs=2)
                          for _ in range(TC)]
                    for q in range(FT // 2):
                        for t in range(TC):
                            nc.tensor.matmul(yp[t][:],
                                             w2[:, 2 * q:2 * q + 2, ts(md, 128)],
                                             h8[:, 2 * q:2 * q + 2, ts(t, 512)],
                                             start=q == 0, stop=False,
                                             perf_mode=PM,
                                             skip_group_check=True)
                    for t in range(TC):
                        nc.tensor.matmul(yp[t][:], b2[:, ts(md, 128)],
                                         onesrow[:], start=False, stop=True,
                                         skip_group_check=True)
                        nc.vector.scalar_tensor_tensor(
                            x[md][:, ts(t, 512)], yp[t][:], 0.5 / SW,
                            x[md][:, ts(t, 512)], op0=OP.mult, op1=OP.add)

        # =========================== program ===============================
        stage = [0]
        def go():
            stage[0] += 1
            return STAGES >= stage[0]

        if go():  # S1 ffn1
            ffn("ffm")

        # ---------------- attention ---------------------------------------
        layer_norm()
        with tc.tile_pool(name="attn", bufs=1) as ap:
            q_u = [ap.tile([128, T], bf16, tag=f"qu{j}", name=f"qu{j}", bufs=1) for j in range(DT)]
            q_v = [ap.tile([128, T], bf16, tag=f"qv{j}", name=f"qv{j}", bufs=1) for j in range(DT)]
            ksb = [ap.tile([128, T], bf16, tag=f"k{j}", name=f"k{j}", bufs=1) for j in range(DT)]
            vT = [ap.tile([128, D], bf16, tag=f"vT{t}", name=f"vT{t}", bufs=1) for t in range(8)]
            att8 = ap.tile([128, DT, T], f8, tag="att8", name="att8", bufs=1)
            bqu, bqv, bva = wt["bqu"], wt["bqv"], wt["bva"]
            win8 = wt["win8"]

            if go():  # S2 qkv
                with tc.tile_pool(name="qkvps", bufs=2, space="PSUM") as qps:
                    for mf in range(2 * DT):          # q then k channel tiles
                        for t in range(TC):
                            qp = qps.tile([128, 512], f32, tag="qp", name="qp", bufs=2)
                            for p in range(DT // 2):
                                nc.tensor.matmul(qp[:],
                                                 win8[:, 2 * p:2 * p + 2, ts(mf, 128)],
                                                 xln8[:, 2 * p:2 * p + 2, ts(t, 512)],
                                                 start=p == 0, stop=p == DT // 2 - 1,
                                                 perf_mode=PM)
                            if mf < DT:
                                nc.scalar.activation(q_u[mf][:, ts(t, 512)], qp[:],
                                                     AF.Identity, scale=1.0 / SW,
                                                     bias=bqu[:, mf:mf + 1])
                                nc.scalar.activation(q_v[mf][:, ts(t, 512)], qp[:],
                                                     AF.Identity, scale=1.0 / SW,
                                                     bias=bqv[:, mf:mf + 1])
                            else:
                                nc.scalar.activation(ksb[mf - DT][:, ts(t, 512)],
                                                     qp[:], AF.Identity,
                                                     scale=1.0 / SW)
                    for tt in range(8):               # v, transposed layout
                        vp = qps.tile([128, 512], f32, tag="qp", name="qp", bufs=2)
                        for p in range(DT // 2):
                            nc.tensor.matmul(vp[:],
                                             xln8[:, 2 * p:2 * p + 2, ts(tt, 128)],
                                             win8[:, 2 * p:2 * p + 2, 2 * D:3 * D],
                                             start=p == 0, stop=p == DT // 2 - 1,
                                             perf_mode=PM)
                        nc.scalar.activation(vT[tt][:], vp[:], AF.Identity,
                                             scale=1.0 / SW)

            if go():  # S3 scores
                with tc.tile_pool(name="sc", bufs=1) as sc, \
                     tc.tile_pool(name="scps", bufs=1, space="PSUM") as scps:
                    for n in range(NB):
                        for hpair in range(H // 2):
                            aos = [scps.tile([128, 512], f32, tag="ao", name="ao",
                                             bufs=2) for _ in range(2)]
                            for h2 in range(2):
                                ao = aos[h2]
                                h_ = 2 * hpair + h2
                                mt, base = h_ // 2, (h_ % 2) * 64
                                qu_h = q_u[mt][base:base + 64, :]
                                qv_h = q_v[mt][base:base + 64, :]
                                k_h = ksb[mt][base:base + 64, :]
                                p_h = pT[mt][base:base + 64, :]
                                probs = []
                                for bl in range(4):
                                    lsl = ts(n * 4 + bl, 128)
                                    acp = scps.tile([128, 512], f32, tag="ac", name="ac", bufs=2)
                                    nc.tensor.matmul(acp[:], qu_h[:, lsl],
                                                     k_h[:, ts(n, 512)],
                                                     start=True, stop=True)
                                    bdp = scps.tile([128, 640], f32, tag="bd", name="bd", bufs=1)
                                    c0 = 384 - 128 * bl
                                    nc.tensor.matmul(bdp[:, 0:512], qv_h[:, lsl],
                                                     p_h[:, c0:c0 + 512],
                                                     start=True, stop=True)
                                    nc.tensor.matmul(bdp[:, 512:640], qv_h[:, lsl],
                                                     p_h[:, c0 + 512:c0 + 640],
                                                     start=True, stop=True)
                                    band = sc.tile([128, 640], f32, tag="band", name="band", bufs=3)
                                    nc.vector.tensor_copy(band[:], bdp[:])
                                    bds = sc.tile([128, 512], f32, tag="bds", name="bds", bufs=3)
                                    nc.sync.dma_start(
                                        bds[:],
                                        bass.AP(band.tensor, 127, [[639, 128], [1, 512]]))
                                    S = sc.tile([128, 512], f32, tag="S", name="S", bufs=3)
                                    nc.vector.tensor_add(S[:], acp[:], bds[:])
                                    pr = sc.tile([128, 512], bf16, tag="pr", name="pr", bufs=5)
                                    z = sc.tile([128, 1], f32, tag="z", name="z", bufs=4)
                                    nc.scalar.activation(pr[:], S[:], AF.Exp,
                                                         scale=SCALE, accum_out=z[:])
                                    r = sc.tile([128, 1], f32, tag="r", name="r", bufs=4)
                                    nc.vector.reciprocal(r[:], z[:])
                                    prn = sc.tile([128, 512], bf16, tag="prn",
                                                  name="prn", bufs=5)
                                    nc.vector.tensor_scalar_mul(prn[:], pr[:], r[:])
                                    probs.append(prn)
                                for sb in range(4):
                                    tp = scps.tile([128, 512], f32, tag="tp", name="tp", bufs=2)
                                    for bl in range(4):
                                        nc.tensor.matmul(tp[:, ts(bl, 128)],
                                                         probs[bl][:, ts(sb, 128)],
                                                         idb[:],
                                                         start=True, stop=True)
                                    pt = sc.tile([128, 512], bf16, tag="pt", name="pt", bufs=4)
                                    (nc.vector.tensor_copy if sb % 2 else
                                     nc.scalar.copy)(pt[:], tp[:])
                                    nc.tensor.matmul(
                                        ao[:],
                                        vT[n * 4 + sb][:, ts(hpair, 128)],
                                        pt[:], start=sb == 0, stop=sb == 3,
                                        skip_group_check=True)
                            for h2 in range(2):
                                b0 = h2 * 64
                                nc.scalar.activation(
                                    att8[b0:b0 + 64, hpair, ts(n, 512)],
                                    aos[h2][b0:b0 + 64, :], AF.Identity,
                                    bias=bva[b0:b0 + 64, hpair:hpair + 1])

            if go():  # S4 out_proj
                wo8, bo = wt["wo8"], wt["bo"]
                with tc.tile_pool(name="wops", bufs=2, space="PSUM") as wps:
                    for md in range(DT):
                        for t in range(TC):
                            op_ = wps.tile([128, 512], f32, tag="op", name="op", bufs=2)
                            for p in range(DT // 2):
                                nc.tensor.matmul(op_[:],
                                                 wo8[:, 2 * p:2 * p + 2, ts(md, 128)],
                                                 att8[:, 2 * p:2 * p + 2, ts(t, 512)],
                                                 start=p == 0, stop=False,
                                                 perf_mode=PM,
                                                 skip_group_check=True)
                            nc.tensor.matmul(op_[:], bo[:, ts(md, 128)],
                                             onesrow[:], start=False, stop=True,
                                             skip_group_check=True)
                            nc.vector.scalar_tensor_tensor(
                                x[md][:, ts(t, 512)], op_[:], 1.0 / SW,
                                x[md][:, ts(t, 512)], op0=OP.mult, op1=OP.add)

        # ---------------- conv module --------------------------------------
        if go():  # S5 conv
            layer_norm()
            wc18, bc1 = wt["wc18"], wt["bc1"]
            dgc8, bdw = wt["dgc8"], wt["bdw"]
            wc28, bc2 = wt["wc28"], wt["bc2"]
            with tc.tile_pool(name="conv", bufs=1) as cp, \
                 tc.tile_pool(name="cvps", bufs=2, space="PSUM") as cps:
                glu_a = [cp.tile([128, T], bf16, tag=f"ga{j}", name=f"ga{j}", bufs=1) for j in range(DT)]
                glu_g = [cp.tile([128, T], bf16, tag=f"gg{j}", name=f"gg{j}", bufs=1) for j in range(DT)]
                for mf in range(2 * DT):
                    for t in range(TC):
                        cpp = cps.tile([128, 512], f32, tag="c1", name="c1", bufs=2)
                        for p in range(DT // 2):
                            nc.tensor.matmul(cpp[:],
                                             wc18[:, 2 * p:2 * p + 2, ts(mf, 128)],
                                             xln8[:, 2 * p:2 * p + 2, ts(t, 512)],
                                             start=p == 0, stop=p == DT // 2 - 1,
                                             perf_mode=PM)
                        if mf < DT:
                            # a-part pre-scaled by GS (folded into glu product)
                            nc.scalar.activation(glu_a[mf][:, ts(t, 512)], cpp[:],
                                                 AF.Identity, scale=GS / SW,
                                                 bias=bc1[:, mf:mf + 1])
                        else:
                            nc.scalar.activation(glu_g[mf - DT][:, ts(t, 512)],
                                                 cpp[:], AF.Sigmoid,
                                                 scale=1.0 / SW,
                                                 bias=bc1[:, mf:mf + 1])
                glu_pad = [cp.tile([128, PW], f8, tag=f"gp{j}", name=f"gp{j}",
                                   bufs=1) for j in range(DT)]
                glu_out8 = cp.tile([128, DT, T], f8, tag="go8", name="go8", bufs=1)
                for j in range(DT):
                    nc.vector.memset(glu_pad[j][:].bitcast(f32), 0.0)
                    for nb in range(NB):
                        o = PAD + nb * SEG
                        nc.vector.tensor_mul(glu_pad[j][:, o:o + L],
                                             glu_a[j][:, ts(nb, L)],
                                             glu_g[j][:, ts(nb, L)])
                for j in range(DT):
                    ccs = [cps.tile([128, 512], f32, tag="dw", name="dw", bufs=4)
                           for _ in range(NB)]
                    for kp in range(16):              # 15 DR tap-pairs + tap 30
                        k = 2 * kp
                        for nb in range(NB):
                            rhs2 = bass.AP(glu_pad[j].tensor, nb * SEG + k,
                                           [[PW, 128], [1, 2], [1, 512]])
                            rhs1 = bass.AP(glu_pad[j].tensor, nb * SEG + k,
                                           [[PW, 128], [1, 512]])
                            if kp < 15:
                                nc.tensor.matmul(ccs[nb][:],
                                                 dgc8[:, j, k:k + 2, :], rhs2,
                                                 start=kp == 0, stop=False,
                                                 perf_mode=PM,
                                                 skip_group_check=True)
                            else:
                                nc.tensor.matmul(ccs[nb][:],
                                                 dgc8[:, j, KW - 1, :], rhs1,
                                                 start=False, stop=True,
                                                 skip_group_check=True)
                    for nb in range(NB):
                        nc.scalar.activation(glu_out8[:, j, ts(nb, 512)], ccs[nb][:],
                                             AF.Silu, scale=1.0 / (GS * SW),
                                             bias=bdw[:, j:j + 1])
                for md in range(DT):
                    for t in range(TC):
                        op_ = cps.tile([128, 512], f32, tag="c2", name="c2", bufs=2)
                        for p in range(DT // 2):
                            nc.tensor.matmul(op_[:],
                                             wc28[:, 2 * p:2 * p + 2, ts(md, 128)],
                                             glu_out8[:, 2 * p:2 * p + 2, ts(t, 512)],
                                             start=p == 0, stop=False,
                                             perf_mode=PM,
                                             skip_group_check=True)
                        nc.tensor.matmul(op_[:], bc2[:, ts(md, 128)],
                                         onesrow[:], start=False, stop=True,
                                         skip_group_check=True)
                        nc.vector.scalar_tensor_tensor(
                            x[md][:, ts(t, 512)], op_[:], 1.0 / SW,
                            x[md][:, ts(t, 512)], op0=OP.mult, op1=OP.add)

        if go():  # S6 ffn2+final
            ffn("ff")

            # ---------------- final layernorm + output ---------------------
            gf, bf_ = wt["gf"], wt["bf"]
            with tc.tile_pool(name="fin", bufs=1) as fin:
                outs = []
                for j in range(DT):
                    o = fin.tile([128, T], f32, tag=f"fo{j}", name=f"fo{j}", bufs=1)
                    outs.append(o[:])
                layer_norm(outs=outs)
                for j in range(DT):
                    nc.scalar.activation(outs[j], outs[j], AF.Identity,
                                         bias=bf_[:, j:j + 1], scale=gf[:, j:j + 1])
                    nc.sync.dma_start(y_d[ts(j, 128), :], outs[j])


# ------------------------------------------------------------ host side ---
def _col128(b):
    """[n*128] -> [128, n] column-bias layout."""
    n = b.size // 128
    return np.ascontiguousarray(b.reshape(n, 128).T).astype(np.float32)


def _f8(a):
    import ml_dtypes
    return np.ascontiguousarray(a).astype(ml_dtypes.float8_e4m3).view(np.uint8)


def _f8ktiles(wT):
    """[K, M] f32 -> [128, K//128, M] fp8e4 (k-subtile-interleaved, as uint8)."""
    K, M = wT.shape
    return _f8(wT.reshape(K // 128, 128, M).transpose(1, 0, 2))


def _bf16(a):
    import ml_dtypes
    return np.ascontiguousarray(a).astype(ml_dtypes.bfloat16).view(np.uint16)


def _prep(inputs):
    i = {k: np.asarray(v, np.float32) for k, v in inputs.items()}
    w = {}
    # host-side pos projection: p = pos_emb[0] @ pos_w.T, laid out [D, 1024]
    p = (i["pos_emb"][0] @ i["pos_w"].T).T            # [D, 2L-1]
    pT = np.zeros((D, 1024), np.float32)
    pT[:, :2 * L - 1] = p
    w["pT"] = _bf16(pT)

    for mod, pre in (("ffm", "ffm"), ("ff", "ff")):
        g, b = i[f"ln_{pre}_g"], i[f"ln_{pre}_b"]
        w[f"w1f8_{mod}"] = _f8ktiles((i[f"{pre}_w1"] * g).T * SW)
        w[f"b1_{mod}"] = _col128(i[f"{pre}_b1"] + i[f"{pre}_w1"] @ b)
        w[f"w2f8_{mod}"] = _f8ktiles(i[f"{pre}_w2"].T * SW)
        w[f"b2_{mod}"] = (i[f"{pre}_b2"] * SW).reshape(1, D).copy()

    g, b = i["ln_mha_g"], i["ln_mha_b"]
    win = i["in_proj_w"] * g
    binp = i["in_proj_b"] + i["in_proj_w"] @ b
    w["win8"] = _f8ktiles(win.T * SW)
    w["bqu"] = _col128(binp[0:D] + i["pos_bias_u"].reshape(D))
    w["bqv"] = _col128(binp[0:D] + i["pos_bias_v"].reshape(D))
    w["bva"] = _col128(binp[2 * D:3 * D])
    w["wo8"] = _f8ktiles(i["out_proj_w"].T * SW)
    w["bo"] = (i["out_proj_b"] * SW).reshape(1, D).copy()

    g, b = i["ln_conv_g"], i["ln_conv_b"]
    w["wc18"] = _f8ktiles((i["conv_pw1_w"] * g).T * SW)
    bc1 = _col128(i["conv_pw1_b"] + i["conv_pw1_w"] @ b)   # [128, 2*DT]
    bc1[:, :DT] *= GS                                       # a-part pre-scale
    w["bc1"] = bc1
    alpha = i["bn_gamma"] / np.sqrt(i["bn_var"] + EPS)
    wdw = i["conv_dw_w"] * alpha[:, None] * SW              # [512, 31]
    # diagonal planes: dgc8[p, j, k, m] = (p == m) * wdw[j*128+p, k]
    dgc = np.zeros((128, DT, KW, 128), np.float32)
    pp = np.arange(128)
    for j in range(DT):
        for k in range(KW):
            dgc[pp, j, k, pp] = wdw[j * 128 + pp, k]
    w["dgc8"] = _f8(dgc)
    w["bdw"] = _col128((i["conv_dw_b"] - i["bn_mean"]) * alpha + i["bn_beta"])
    w["wc28"] = _f8ktiles(i["conv_pw2_w"].T * SW)
    w["bc2"] = (i["conv_pw2_b"] * SW).reshape(1, D).copy()
    w["gf"] = _col128(i["ln_final_g"])
    w["bf"] = _col128(i["ln_final_b"])
    return i, w


_NC_CACHE = []


def get_nc():
    if not _NC_CACHE:
        _NC_CACHE.append(_build())
    return _NC_CACHE[0]


def make_in_maps(inputs):
    i, w = _prep(inputs)
    in_maps = []
    for c in range(NCORES):
        m = dict(w)
        # src (L, N, D) -> [D, T] with t = n*L + l
        m["x"] = np.ascontiguousarray(
            i["src"][:, NB * c:NB * (c + 1), :].transpose(2, 1, 0).reshape(D, T))
        in_maps.append(m)
    return in_maps


def assemble(results):
    out = np.empty((L, N, D), np.float32)
    for c in range(NCORES):
        y = results[c]["y"]                             # [D, T]
        for nb in range(NB):
            out[:, NB * c + nb, :] = y[:, nb * L:(nb + 1) * L].T
    return out


def kernel(**inputs):
    nc = get_nc()
    res = run_bass_kernel_spmd(nc, make_in_maps(inputs), list(range(NCORES)))
    return assemble(res.results)
